# revision 25
# baseline (speedup 1.0000x reference)
"""CLIP text transformer with prompt tuning on 8 TRN2 NeuronCores.

Data-parallel over batch: each core runs the full 12-layer transformer on 16
sequences. Activations live in SBUF for the whole forward pass; weights are
folded (LN gains, qk scale, bv@Wo+bo) on the host and streamed per layer in
bf16.

The layer body is software-pipelined to keep the PE array streaming real
matmuls continuously (the HAM clock governor halves the PE clock after
~3.4us without matmul activity, and transposes don't count):
  - phase L: LN1 stats/apply/transposes interleaved with Q/K token-chunks,
    V sequences, and the previous layer's deferred W2 tail tiles
  - phase A1: all 16 sequences' QK score matmuls + exp + causal mask,
    back-to-back (scores for all sequences parked in SBUF)
  - phase A2: per-sequence AV/normalize/transpose interleaved with Wo,
    LN2, and early W1 blocks (gelus stay grouped after all exps so the
    scalar engine swaps activation tables only twice per layer)
  - phase M: remaining W1 + W2 tiles (last two W2 tiles deferred into the
    next layer's phase L)
"""
import sys

sys.path.insert(0, "/opt/trn_rl_repo")

import json

import ml_dtypes
import numpy as np

import concourse.bass as bass
import concourse.mybir as mybir
import concourse.tile as tile

# ----------------------------------------------------------------------------
# walrus in this container rejects instructions with more than one sync wait /
# update; split them into chains of single-wait Drains before compiling.
import concourse.bass2jax as _b2j

_orig_compile_bir = _b2j.compile_bir_kernel


def _split_sync(bir_bytes):
    bir = json.loads(bir_bytes)
    for fn in bir.get("functions", []):
        for bb in fn.get("blocks", []):
            new = []
            for inst in bb.get("instructions", []):
                si = inst.get("sync_info")
                waits = (si or {}).get("on_wait") or []
                if len(waits) > 1:
                    for i, w in enumerate(waits[:-1]):
                        new.append({
                            "debug": inst.get("debug", 0),
                            "engine": inst["engine"],
                            "ins": [], "outs": [],
                            "name": f"{inst['name']}__w{i}",
                            "opcode": "NoOp",
                            "sync_info": {"on_update": [], "on_wait": [w]},
                        })
                    si["on_wait"] = waits[-1:]
                new.append(inst)
                updates = (si or {}).get("on_update") or []
                if len(updates) > 1:
                    for i, u in enumerate(updates[1:]):
                        new.append({
                            "debug": inst.get("debug", 0),
                            "engine": inst["engine"],
                            "ins": [], "outs": [],
                            "name": f"{inst['name']}__u{i}",
                            "opcode": "NoOp",
                            "sync_info": {"on_update": [u], "on_wait": []},
                        })
                    si["on_update"] = updates[:1]
            bb["instructions"] = new
    return json.dumps(bir).encode()


def _patched_compile_bir(bir_json, tmpdir, neff_name="file.neff"):
    return _orig_compile_bir(_split_sync(bir_json), tmpdir, neff_name)


_b2j.compile_bir_kernel = _patched_compile_bir
# ----------------------------------------------------------------------------

from concourse.bass_utils import run_bass_kernel_spmd

f32 = mybir.dt.float32
bf16 = mybir.dt.bfloat16
AF = mybir.ActivationFunctionType
ALU = mybir.AluOpType

V, P, D, H, NL, FF = 49408, 77, 512, 8, 12, 2048
B, T, NP = 128, 69, 8
DG, DS = 6, 6
EPS = 1e-5
SCALE = 0.125
NCORES = 8
S = T + NP            # 77
BSH = B // NCORES     # 16 sequences per core
NT = BSH * S          # 1232 tokens per core
NTILES = 10           # 1280 padded
DB = D // 128         # 4 feature blocks
FB = FF // 128        # 16 ff blocks
DH = D // H           # 64

QKCH = [(0, 512), (512, 512), (1024, 256)]
W1CH = [(0, 512), (512, 512), (1024, 256)]
# V sequences ready after each Q/K chunk
V_READY = [[b for b in range(BSH)
            if (QKCH[c][0] if c else 0) < (b + 1) * S <= QKCH[c][0] + QKCH[c][1]]
           for c in range(3)]
# number of W1 c0-blocks interleaved into attention phase A2 (keeps the PE
# streaming big matmuls through the AV/normalize tail; costs ~2 extra ACT
# table switches per layer, which the warm PE clock more than pays back)
W1_EARLY = 16


def _bf(x):
    return np.ascontiguousarray(x.astype(ml_dtypes.bfloat16))


def _f32(x):
    return np.ascontiguousarray(x.astype(np.float32))


def build_program(n_layers=NL):
    nc = bass.Bass()
    dp = nc.declare_dram_parameter
    X0 = dp("x0", [128, NTILES, D], f32, isOutput=False)
    PR = dp("prompts", [NL - 1, BSH, NP, D], f32, isOutput=False)
    WQ = dp("wq", [NL, D, D], bf16, isOutput=False)
    WK = dp("wk", [NL, D, D], bf16, isOutput=False)
    WV = dp("wv", [NL, D, D], bf16, isOutput=False)
    WO = dp("wo", [NL, D, D], bf16, isOutput=False)
    W1 = dp("w1", [NL, D, FF], bf16, isOutput=False)
    W2 = dp("w2", [NL, FF, D], bf16, isOutput=False)
    BQ = dp("bq", [NL, 128, DB], f32, isOutput=False)
    BK = dp("bk", [NL, 128, DB], f32, isOutput=False)
    B1 = dp("b1", [NL, 128, FB], f32, isOutput=False)
    BROW = dp("brow", [NL, 2, D], bf16, isOutput=False)  # bv@Wo+bo, b2
    CA = dp("causal", [P, 128], bf16, isOutput=False)    # [tk, tq] mask
    ID = dp("ident", [128, 128], bf16, isOutput=False)
    OUT = dp("out", [128, NTILES, D], f32, isOutput=True)

    from contextlib import ExitStack
    with tile.TileContext(nc) as tc, ExitStack() as ctx:
        consts = ctx.enter_context(tc.tile_pool(name="consts", bufs=1))
        persist = ctx.enter_context(tc.tile_pool(name="persist", bufs=1))
        wpool = ctx.enter_context(tc.tile_pool(name="wpool", bufs=1))
        bp = ctx.enter_context(tc.tile_pool(name="bp", bufs=2))
        hlnp = ctx.enter_context(tc.tile_pool(name="hlnp", bufs=4))
        ptp = ctx.enter_context(tc.tile_pool(name="ptp", bufs=17))
        osbp = ctx.enter_context(tc.tile_pool(name="osbp", bufs=3))
        statp = ctx.enter_context(tc.tile_pool(name="statp", bufs=4))
        pq = ctx.enter_context(tc.tile_pool(name="pq", bufs=2, space="PSUM"))
        pbig = ctx.enter_context(tc.tile_pool(name="pbig", bufs=2, space="PSUM"))

        ident = consts.tile([128, 128], bf16, tag="ident")
        nc.sync.dma_start(out=ident, in_=ID[:, :])
        causal = consts.tile([P, 128], bf16, tag="causal")
        nc.sync.dma_start(out=causal, in_=CA[:, :])
        ones_row = consts.tile([1, 128], bf16, tag="ones")
        nc.gpsimd.memset(ones_row, 1.0)
        eps_t = consts.tile([128, 1], f32, tag="eps")
        nc.gpsimd.memset(eps_t, EPS)

        x_t = persist.tile([128, NTILES, D], f32, tag="x")
        nc.sync.dma_start(out=x_t, in_=X0[:, :, :])
        hfm = persist.tile([128, DB, NTILES * 128], bf16, tag="hfm")
        q_fm = persist.tile([128, DB, NTILES * 128], bf16, tag="qfm")
        k_fm = persist.tile([128, DB, NTILES * 128], bf16, tag="kfm")
        o_fm = persist.tile([128, DB, NTILES * 128], bf16, tag="ofm")
        nc.gpsimd.memset(o_fm[:, :, NT:], 0.0)
        v_tm = persist.tile([128, BSH, H, DH + 1], bf16, tag="v")
        nc.gpsimd.memset(v_tm[0:P, :, :, DH : DH + 1], 1.0)
        h_mlp = persist.tile([128, FB, NTILES * 128], bf16, tag="hmlp")

        def ln_stats(mv, t):
            st = statp.tile([128, 6], f32, tag="bnst")
            nc.vector.bn_stats(out=st, in_=x_t[:, t, :])
            nc.vector.bn_aggr(out=mv[:, t, :], in_=st)

        def ln_rsqrt_batch(mv, rs, t0, t1, mrsneg=None):
            lv = statp.tile([128, 6, 1], f32, tag="lv")
            nc.scalar.activation(out=lv[:, 0 : t1 - t0, :], in_=mv[:, t0:t1, 1:2],
                                 func=AF.Ln, bias=eps_t)
            nc.scalar.activation(out=rs[:, t0:t1, :], in_=lv[:, 0 : t1 - t0, :],
                                 func=AF.Exp, scale=-0.5)
            if mrsneg is not None:
                # -mean * rsqrt, for the ACT-side LN apply
                nc.vector.scalar_tensor_tensor(
                    out=mrsneg[:, t0:t1, :], in0=mv[:, t0:t1, 0:1],
                    scalar=-1.0, in1=rs[:, t0:t1, :],
                    op0=ALU.mult, op1=ALU.mult,
                )

        def ln_apply_transpose(mv, rs, t, mrsneg=None):
            h = hlnp.tile([128, D], bf16, tag="hln")
            if mrsneg is None:
                nc.vector.tensor_scalar(
                    out=h, in0=x_t[:, t, :],
                    scalar1=mv[:, t, 0:1], scalar2=rs[:, t, :],
                    op0=ALU.subtract, op1=ALU.mult,
                )
            else:
                nc.scalar.activation(
                    out=h, in_=x_t[:, t, :], func=AF.Identity,
                    scale=rs[:, t, :], bias=mrsneg[:, t, :],
                )
            po = pq.tile([128, D], bf16, tag="po", bufs=2)
            pov = po.rearrange("p (db c) -> p db c", db=DB)
            for db in range(DB):
                nc.tensor.transpose(
                    pov[:, db, :], h[:, db * 128 : (db + 1) * 128], ident
                )
            nc.scalar.activation(
                out=hfm[:, :, t * 128 : (t + 1) * 128], in_=pov, func=AF.Identity
            )

        deferred_w2 = []

        for li in range(n_layers):
            # ---- prompt replacement for rows landing in tiles 0..7
            # (tiles 8/9 carry the previous layer's deferred W2 residuals and
            #  are patched after those run)
            def prompt_dma(b):
                t0 = b * S + 1
                p0, tl = t0 % 128, t0 // 128
                nc.sync.dma_start(out=x_t[p0 : p0 + NP, tl, :], in_=PR[li - 1, b])

            if li > 0:
                for b in range(BSH - 2):
                    prompt_dma(b)
            bq_t = bp.tile([128, DB], f32, tag="bq")
            nc.sync.dma_start(out=bq_t, in_=BQ[li])
            bk_t = bp.tile([128, DB], f32, tag="bk")
            nc.sync.dma_start(out=bk_t, in_=BK[li])
            b1_t = bp.tile([128, FB], f32, tag="b1")
            nc.sync.dma_start(out=b1_t, in_=B1[li])
            brow_t = bp.tile([1, 2, D], bf16, tag="brow")
            nc.sync.dma_start(out=brow_t, in_=BROW[li])
            wq_t = wpool.tile([128, DB, D], bf16, tag="wq")
            nc.sync.dma_start(out=wq_t, in_=WQ[li].rearrange("(kb p) m -> p kb m", p=128))
            wk_t = wpool.tile([128, DB, D], bf16, tag="wk")
            nc.sync.dma_start(out=wk_t, in_=WK[li].rearrange("(kb p) m -> p kb m", p=128))
            wv_t = wpool.tile([128, DB, D], bf16, tag="wv")
            nc.sync.dma_start(out=wv_t, in_=WV[li].rearrange("(kb p) m -> p kb m", p=128))
            wo_t = wpool.tile([128, DB, D], bf16, tag="wo")
            nc.sync.dma_start(out=wo_t, in_=WO[li].rearrange("(kb p) m -> p kb m", p=128))
            w1_t = wpool.tile([128, DB, FF], bf16, tag="w1")
            nc.sync.dma_start(out=w1_t, in_=W1[li].rearrange("(kb p) m -> p kb m", p=128))
            w2_t = wpool.tile([128, FB, D], bf16, tag="w2")
            nc.sync.dma_start(out=w2_t, in_=W2[li].rearrange("(kb p) m -> p kb m", p=128))

            def qk_chunk(c):
                c0, cw = QKCH[c]
                for m in range(DB):
                    ps = pq.tile([128, D], f32, tag="pq")
                    for k in range(DB):
                        nc.tensor.matmul(
                            ps[:, 0:cw],
                            wq_t[:, k, m * 128 : (m + 1) * 128],
                            hfm[:, k, c0 : c0 + cw],
                            start=(k == 0), stop=(k == DB - 1),
                        )
                    nc.scalar.activation(
                        out=q_fm[:, m, c0 : c0 + cw], in_=ps[:, 0:cw],
                        func=AF.Identity, bias=bq_t[:, m : m + 1],
                    )
                for m in range(DB):
                    ps = pq.tile([128, D], f32, tag="pq")
                    for k in range(DB):
                        nc.tensor.matmul(
                            ps[:, 0:cw],
                            wk_t[:, k, m * 128 : (m + 1) * 128],
                            hfm[:, k, c0 : c0 + cw],
                            start=(k == 0), stop=(k == DB - 1),
                        )
                    nc.vector.tensor_scalar_add(
                        out=k_fm[:, m, c0 : c0 + cw], in0=ps[:, 0:cw],
                        scalar1=bk_t[:, m : m + 1],
                    )
                for b in V_READY[c]:
                    bs = b * S
                    ps = pq.tile([128, D], f32, tag="pq")
                    for k in range(DB):
                        nc.tensor.matmul(
                            ps[0:P, :],
                            hfm[:, k, bs : bs + S],
                            wv_t[:, k, :],
                            start=(k == 0), stop=(k == DB - 1),
                        )
                    nc.vector.tensor_copy(
                        out=v_tm[0:P, b, :, 0:DH],
                        in_=ps[0:P, :].rearrange("p (h d) -> p h d", h=H),
                    )

            # ---- phase L: LN1 + Q/K/V, deferred W2 tail of previous layer
            mv1 = statp.tile([128, NTILES, 2], f32, tag="mv1")
            rs1 = statp.tile([128, NTILES, 1], f32, tag="rs1")
            for t in range(0, 5):
                ln_stats(mv1, t)
            ln_rsqrt_batch(mv1, rs1, 0, 5)
            for t in range(0, 4):
                ln_apply_transpose(mv1, rs1, t)
                if deferred_w2 and t < 2:
                    deferred_w2.pop(0)()
                    if not deferred_w2 and li > 0:
                        prompt_dma(BSH - 2)
                        prompt_dma(BSH - 1)
            qk_chunk(0)
            for t in range(5, 10):
                ln_stats(mv1, t)
            ln_rsqrt_batch(mv1, rs1, 5, 10)
            for t in range(4, 8):
                ln_apply_transpose(mv1, rs1, t)
            qk_chunk(1)
            for t in range(8, 10):
                ln_apply_transpose(mv1, rs1, t)
            qk_chunk(2)

            # ---- phase A2: AV + output transposes, interleaved with Wo/LN2/W1
            mv2 = statp.tile([128, NTILES, 2], f32, tag="mv2")
            rs2 = statp.tile([128, NTILES, 1], f32, tag="rs2")
            wo_done = 0
            w1_early = 0

            def wo_tile(t):
                ps = pq.tile([128, D], f32, tag="pq")
                for k in range(DB):
                    nc.tensor.matmul(
                        ps[:, :],
                        o_fm[:, k, t * 128 : (t + 1) * 128],
                        wo_t[:, k, :],
                        start=(k == 0), stop=False,
                    )
                nc.tensor.matmul(
                    ps[:, :], ones_row[0:1, :], brow_t[0:1, 0, :],
                    start=False, stop=True,
                )
                nc.vector.tensor_add(x_t[:, t, :], x_t[:, t, :], ps[:, :])
                ln_stats(mv2, t)

            def w1_block(c0, cw, m):
                ps = pq.tile([128, D], f32, tag="pq")
                for k in range(DB):
                    nc.tensor.matmul(
                        ps[:, 0:cw],
                        w1_t[:, k, m * 128 : (m + 1) * 128],
                        hfm[:, k, c0 : c0 + cw],
                        start=(k == 0), stop=(k == DB - 1),
                    )
                nc.scalar.activation(
                    out=h_mlp[:, m, c0 : c0 + cw], in_=ps[:, 0:cw],
                    func=AF.Gelu_apprx_sigmoid, bias=b1_t[:, m : m + 1],
                )

            pts = {}
            for j in range(BSH + 2):
                if j < BSH:
                    bs = j * S
                    sc = pbig.tile([128, 1024], f32, tag="pbig")
                    scv = sc.rearrange("p (e d q) -> p e d q", e=2, q=128)
                    for dbl in range(DB):
                        nc.tensor.matmul(
                            scv[0:P, 0, dbl, 0:P],
                            k_fm[0:64, dbl, bs : bs + S],
                            q_fm[0:64, dbl, bs : bs + S],
                            start=True, stop=True,
                        )
                        nc.tensor.matmul(
                            scv[0:P, 1, dbl, 0:P],
                            k_fm[64:128, dbl, bs : bs + S],
                            q_fm[64:128, dbl, bs : bs + S],
                            start=True, stop=True,
                        )
                    pt = ptp.tile([128, 2, DB, 80], bf16, tag="pt")
                    for e in range(2):
                        nc.scalar.activation(
                            out=pt[0:P, e, :, 0:P], in_=scv[0:P, e, :, 0:P],
                            func=AF.Exp,
                        )
                        nc.vector.tensor_mul(
                            pt[0:P, e, :, 0:P], pt[0:P, e, :, 0:P],
                            causal[0:P, 0:P].unsqueeze(1).broadcast_to((P, DB, P)),
                        )
                    pts[j] = pt
                if j < 2:
                    continue
                b = j - 2
                bs = b * S
                pt = pts.pop(b)
                otf = pbig.tile([128, 1024], f32, tag="pbig")
                ot = otf.rearrange("p (h c) -> p h c", c=128)
                for h in range(H):
                    nc.tensor.matmul(
                        ot[0:P, h, 0 : DH + 1],
                        pt[0:P, h % 2, h // 2, 0:P],
                        v_tm[0:P, b, h, :],
                        start=True, stop=True,
                    )
                rec = statp.tile([128, H], f32, tag="rec")
                nc.vector.reciprocal(
                    out=rec[0:P, :].unsqueeze(2), in_=ot[0:P, :, DH : DH + 1]
                )
                osb = osbp.tile([128, D], bf16, tag="osb")
                nc.vector.tensor_mul(
                    osb[0:P, :].rearrange("p (h d) -> p h d", h=H),
                    ot[0:P, :, 0:DH],
                    rec[0:P, :].unsqueeze(2).broadcast_to((P, H, DH)),
                )
                po = pq.tile([128, D], bf16, tag="po", bufs=2)
                pov = po.rearrange("p (db c) -> p db c", db=DB)
                for db in range(DB):
                    nc.tensor.transpose(
                        pov[:, db, 0:P], osb[0:P, db * 128 : (db + 1) * 128],
                        ident[0:P, 0:P],
                    )
                nc.scalar.activation(
                    out=o_fm[:, :, bs : bs + S], in_=pov[:, :, 0:P],
                    func=AF.Identity,
                )

                while (wo_done + 1) * 128 <= (b + 1) * S:
                    wo_tile(wo_done)
                    wo_done += 1
                    if wo_done == 5:
                        ln_rsqrt_batch(mv2, rs2, 0, 5)
                        for t in range(0, 5):
                            ln_apply_transpose(mv2, rs2, t)
                for _ in range(2):
                    if wo_done >= 5 and w1_early < W1_EARLY:
                        w1_block(W1CH[0][0], W1CH[0][1], w1_early)
                        w1_early += 1

            for t in range(wo_done, NTILES):
                wo_tile(t)
            ln_rsqrt_batch(mv2, rs2, 5, NTILES)
            for t in range(5, NTILES):
                ln_apply_transpose(mv2, rs2, t)

            # ---- phase M: remaining W1 + W2 (last two tiles deferred)
            def w2_tile(t, w2_ref, brow_ref):
                def emit():
                    ps = pq.tile([128, D], f32, tag="pq")
                    for k in range(FB):
                        nc.tensor.matmul(
                            ps[:, :],
                            h_mlp[:, k, t * 128 : (t + 1) * 128],
                            w2_ref[:, k, :],
                            start=(k == 0), stop=False,
                        )
                    nc.tensor.matmul(
                        ps[:, :], ones_row[0:1, :], brow_ref[0:1, 1, :],
                        start=False, stop=True,
                    )
                    nc.vector.tensor_add(x_t[:, t, :], x_t[:, t, :], ps[:, :])
                return emit

            W2T = [range(0, 4), range(4, 8), range(8, 10)]
            last = li == n_layers - 1
            for ci, (c0, cw) in enumerate(W1CH):
                for m in range(w1_early if ci == 0 else 0, FB):
                    w1_block(c0, cw, m)
                for t in W2T[ci]:
                    if ci == 2 and not last:
                        deferred_w2.append(w2_tile(t, w2_t, brow_t))
                    else:
                        w2_tile(t, w2_t, brow_t)()

        nc.sync.dma_start(out=OUT[:, :, :], in_=x_t)
    return nc


_NC_CACHE = None
_LAST_IN_MAPS = None


def _get_nc():
    global _NC_CACHE
    if _NC_CACHE is None:
        _NC_CACHE = build_program()
    return _NC_CACHE


def kernel(text_tokens, attn_mask, g_prompt, s_prompt, token_emb, pos_emb,
           ln1_g, ln1_b, Wq, bq, Wk, bk, Wv, bv, Wo, bo,
           ln2_g, ln2_b, W1, b1, W2, b2, lnf_g, lnf_b):
    text_tokens = np.asarray(text_tokens)
    attn_mask = np.asarray(attn_mask)
    assert np.all(np.asarray(attn_mask) == 1), "kernel assumes all-ones attn_mask"
    fp = lambda a: np.asarray(a, dtype=np.float32)
    g_prompt, s_prompt = fp(g_prompt), fp(s_prompt)
    token_emb, pos_emb = fp(token_emb), fp(pos_emb)
    ln1_g, ln1_b, ln2_g, ln2_b = fp(ln1_g), fp(ln1_b), fp(ln2_g), fp(ln2_b)
    Wq, Wk, Wv, Wo, W1, W2 = fp(Wq), fp(Wk), fp(Wv), fp(Wo), fp(W1), fp(W2)
    bq, bk, bv, bo, b1, b2 = fp(bq), fp(bk), fp(bv), fp(bo), fp(b1), fp(b2)
    lnf_g, lnf_b = fp(lnf_g), fp(lnf_b)

    # ---- host-side input prep
    emb = token_emb[text_tokens]                                  # [B, T, D]
    x0 = np.concatenate([emb[:, :1], g_prompt[:, 0], emb[:, 1:]], axis=1)
    x0 = x0 + pos_emb[None, :S]                                   # [B, S, D]

    # fold LN gains/scale into weights; fold bv through Wo into a bias row
    wq_e = _bf(ln1_g[:, :, None] * Wq * SCALE)
    wk_e = _bf(ln1_g[:, :, None] * Wk)
    wv_e = _bf(ln1_g[:, :, None] * Wv)
    wo_e = _bf(Wo)
    w1_e = _bf(ln2_g[:, :, None] * W1)
    w2_e = _bf(W2)
    bq_e = (bq + np.einsum("ld,ldm->lm", ln1_b, Wq)) * SCALE      # [NL, D]
    bk_e = bk + np.einsum("ld,ldm->lm", ln1_b, Wk)
    bv_e = bv + np.einsum("ld,ldm->lm", ln1_b, Wv)
    b1_e = b1 + np.einsum("ld,ldm->lm", ln2_b, W1)
    crow = np.einsum("ld,ldm->lm", bv_e, Wo) + bo                 # bv@Wo + bo
    bq_dev = _f32(bq_e.reshape(NL, DB, 128).transpose(0, 2, 1))
    bk_dev = _f32(bk_e.reshape(NL, DB, 128).transpose(0, 2, 1))
    b1_dev = _f32(b1_e.reshape(NL, FB, 128).transpose(0, 2, 1))
    brow = _bf(np.stack([crow, np.broadcast_to(b2, (NL, D))], axis=1))

    causalT = np.zeros((P, 128), np.float32)                      # [tk, tq]
    causalT[:, :P] = np.triu(np.ones((P, P), np.float32))
    ident = _bf(np.eye(128, dtype=np.float32))

    in_maps = []
    for c in range(NCORES):
        sl = slice(c * BSH, (c + 1) * BSH)
        flat = x0[sl].reshape(NT, D)
        flat = np.concatenate([flat, np.zeros((NTILES * 128 - NT, D), np.float32)])
        x0_dev = _f32(flat.reshape(NTILES, 128, D).transpose(1, 0, 2))
        prompts = np.stack(
            [g_prompt[sl, i] if i < DG else s_prompt[sl, i - (NL - DS)]
             for i in range(1, NL)]
        )
        in_maps.append({
            "x0": x0_dev, "prompts": _f32(prompts),
            "wq": wq_e, "wk": wk_e, "wv": wv_e, "wo": wo_e,
            "w1": w1_e, "w2": w2_e,
            "bq": bq_dev, "bk": bk_dev, "b1": b1_dev, "brow": brow,
            "causal": _bf(causalT), "ident": ident,
        })

    nc = _get_nc()
    global _LAST_IN_MAPS
    _LAST_IN_MAPS = in_maps
    res = run_bass_kernel_spmd(nc, in_maps, core_ids=list(range(NCORES)))

    # ---- host-side epilogue: final LN + EOT gather
    idx = np.argmax(text_tokens, axis=-1) + NP                    # [B]
    out = np.empty((B, D), np.float32)
    for c in range(NCORES):
        xr = res.results[c]["out"].transpose(1, 0, 2).reshape(NTILES * 128, D)
        for b in range(BSH):
            row = xr[(b * S) + idx[c * BSH + b]]
            m = row.mean()
            v = ((row - m) ** 2).mean()
            out[c * BSH + b] = (row - m) / np.sqrt(v + EPS) * lnf_g + lnf_b
    return out


# revision 26
# speedup vs baseline: 1.1414x; 1.1414x over previous
"""CLIP text transformer with prompt tuning on 8 TRN2 NeuronCores.

Data-parallel over batch: each core runs the full 12-layer transformer on 16
sequences. Activations live in SBUF for the whole forward pass; weights are
folded (LN gains, qk scale, bv@Wo+bo) on the host and streamed per layer in
bf16.

The layer body is software-pipelined to keep the PE array streaming real
matmuls continuously (the HAM clock governor halves the PE clock after
~3.4us without matmul activity, and transposes don't count):
  - phase L: LN1 stats/apply/transposes interleaved with Q/K token-chunks,
    V sequences, and the previous layer's deferred W2 tail tiles
  - phase A1: all 16 sequences' QK score matmuls + exp + causal mask,
    back-to-back (scores for all sequences parked in SBUF)
  - phase A2: per-sequence AV/normalize/transpose interleaved with Wo,
    LN2, and early W1 blocks (gelus stay grouped after all exps so the
    scalar engine swaps activation tables only twice per layer)
  - phase M: remaining W1 + W2 tiles (last two W2 tiles deferred into the
    next layer's phase L)
"""
import sys

sys.path.insert(0, "/opt/trn_rl_repo")

import json

import ml_dtypes
import numpy as np

import concourse.bass as bass
import concourse.mybir as mybir
import concourse.tile as tile

# ----------------------------------------------------------------------------
# walrus in this container rejects instructions with more than one sync wait /
# update; split them into chains of single-wait Drains before compiling.
import concourse.bass2jax as _b2j

_orig_compile_bir = _b2j.compile_bir_kernel


def _split_sync(bir_bytes):
    bir = json.loads(bir_bytes)
    for fn in bir.get("functions", []):
        for bb in fn.get("blocks", []):
            new = []
            for inst in bb.get("instructions", []):
                si = inst.get("sync_info")
                waits = (si or {}).get("on_wait") or []
                if len(waits) > 1:
                    for i, w in enumerate(waits[:-1]):
                        new.append({
                            "debug": inst.get("debug", 0),
                            "engine": inst["engine"],
                            "ins": [], "outs": [],
                            "name": f"{inst['name']}__w{i}",
                            "opcode": "NoOp",
                            "sync_info": {"on_update": [], "on_wait": [w]},
                        })
                    si["on_wait"] = waits[-1:]
                new.append(inst)
                updates = (si or {}).get("on_update") or []
                if len(updates) > 1:
                    for i, u in enumerate(updates[1:]):
                        new.append({
                            "debug": inst.get("debug", 0),
                            "engine": inst["engine"],
                            "ins": [], "outs": [],
                            "name": f"{inst['name']}__u{i}",
                            "opcode": "NoOp",
                            "sync_info": {"on_update": [u], "on_wait": []},
                        })
                    si["on_update"] = updates[:1]
            bb["instructions"] = new
    return json.dumps(bir).encode()


def _patched_compile_bir(bir_json, tmpdir, neff_name="file.neff"):
    return _orig_compile_bir(_split_sync(bir_json), tmpdir, neff_name)


_b2j.compile_bir_kernel = _patched_compile_bir
# ----------------------------------------------------------------------------

from concourse.bass_utils import run_bass_kernel_spmd

f32 = mybir.dt.float32
bf16 = mybir.dt.bfloat16
AF = mybir.ActivationFunctionType
ALU = mybir.AluOpType

V, P, D, H, NL, FF = 49408, 77, 512, 8, 12, 2048
B, T, NP = 128, 69, 8
DG, DS = 6, 6
EPS = 1e-5
SCALE = 0.125
NCORES = 8
S = T + NP            # 77
BSH = B // NCORES     # 16 sequences per core
NT = BSH * S          # 1232 tokens per core
NTILES = 10           # 1280 padded
DB = D // 128         # 4 feature blocks
FB = FF // 128        # 16 ff blocks
DH = D // H           # 64

QKCH = [(0, 512), (512, 512), (1024, 256)]
W1CH = [(0, 512), (512, 512), (1024, 256)]
# V sequences ready after each Q/K chunk
V_READY = [[b for b in range(BSH)
            if (QKCH[c][0] if c else 0) < (b + 1) * S <= QKCH[c][0] + QKCH[c][1]]
           for c in range(3)]
# number of W1 c0-blocks interleaved into attention phase A2 (keeps the PE
# streaming big matmuls through the AV/normalize tail; costs ~2 extra ACT
# table switches per layer, which the warm PE clock more than pays back)
W1_EARLY = 16


def _bf(x):
    return np.ascontiguousarray(x.astype(ml_dtypes.bfloat16))


def _f32(x):
    return np.ascontiguousarray(x.astype(np.float32))


def build_program(n_layers=NL):
    nc = bass.Bass()
    dp = nc.declare_dram_parameter
    X0 = dp("x0", [128, NTILES, D], f32, isOutput=False)
    PR = dp("prompts", [NL - 1, BSH, NP, D], f32, isOutput=False)
    WQ = dp("wq", [NL, D, D], bf16, isOutput=False)
    WK = dp("wk", [NL, D, D], bf16, isOutput=False)
    WV = dp("wv", [NL, D, D], bf16, isOutput=False)
    WO = dp("wo", [NL, D, D], bf16, isOutput=False)
    W1 = dp("w1", [NL, D, FF], bf16, isOutput=False)
    W2 = dp("w2", [NL, FF, D], bf16, isOutput=False)
    BQ = dp("bq", [NL, 128, DB], f32, isOutput=False)
    BK = dp("bk", [NL, 128, DB], f32, isOutput=False)
    B1 = dp("b1", [NL, 128, FB], f32, isOutput=False)
    BROW = dp("brow", [NL, 2, D], bf16, isOutput=False)  # bv@Wo+bo, b2
    CA = dp("causal", [P, 128], bf16, isOutput=False)    # [tk, tq] mask
    ID = dp("ident", [128, 128], bf16, isOutput=False)
    OUT = dp("out", [128, NTILES, D], f32, isOutput=True)

    from contextlib import ExitStack
    with tile.TileContext(nc) as tc, ExitStack() as ctx:
        consts = ctx.enter_context(tc.tile_pool(name="consts", bufs=1))
        persist = ctx.enter_context(tc.tile_pool(name="persist", bufs=1))
        wpool = ctx.enter_context(tc.tile_pool(name="wpool", bufs=1))
        bp = ctx.enter_context(tc.tile_pool(name="bp", bufs=2))
        hlnp = ctx.enter_context(tc.tile_pool(name="hlnp", bufs=4))
        ptp = ctx.enter_context(tc.tile_pool(name="ptp", bufs=17))
        osbp = ctx.enter_context(tc.tile_pool(name="osbp", bufs=3))
        statp = ctx.enter_context(tc.tile_pool(name="statp", bufs=4))
        pq = ctx.enter_context(tc.tile_pool(name="pq", bufs=2, space="PSUM"))
        pbig = ctx.enter_context(tc.tile_pool(name="pbig", bufs=2, space="PSUM"))

        ident = consts.tile([128, 128], bf16, tag="ident")
        nc.sync.dma_start(out=ident, in_=ID[:, :])
        causal = consts.tile([P, 128], bf16, tag="causal")
        nc.sync.dma_start(out=causal, in_=CA[:, :])
        ones_row = consts.tile([1, 128], bf16, tag="ones")
        nc.gpsimd.memset(ones_row, 1.0)
        eps_t = consts.tile([128, 1], f32, tag="eps")
        nc.gpsimd.memset(eps_t, EPS)

        x_t = persist.tile([128, NTILES, D], f32, tag="x")
        nc.sync.dma_start(out=x_t, in_=X0[:, :, :])
        hfm = persist.tile([128, DB, NTILES * 128], bf16, tag="hfm")
        q_fm = persist.tile([128, DB, NTILES * 128], bf16, tag="qfm")
        k_fm = persist.tile([128, DB, NTILES * 128], bf16, tag="kfm")
        o_fm = persist.tile([128, DB, NTILES * 128], bf16, tag="ofm")
        nc.gpsimd.memset(o_fm[:, :, NT:], 0.0)
        v_tm = persist.tile([128, BSH, H, DH + 1], bf16, tag="v")
        nc.gpsimd.memset(v_tm[0:P, :, :, DH : DH + 1], 1.0)
        h_mlp = persist.tile([128, FB, NTILES * 128], bf16, tag="hmlp")

        def ln_stats(mv, t):
            st = statp.tile([128, 6], f32, tag="bnst")
            nc.vector.bn_stats(out=st, in_=x_t[:, t, :])
            nc.vector.bn_aggr(out=mv[:, t, :], in_=st)

        def ln_rsqrt_batch(mv, rs, t0, t1, mrsneg=None):
            lv = statp.tile([128, 6, 1], f32, tag="lv")
            nc.scalar.activation(out=lv[:, 0 : t1 - t0, :], in_=mv[:, t0:t1, 1:2],
                                 func=AF.Ln, bias=eps_t)
            nc.scalar.activation(out=rs[:, t0:t1, :], in_=lv[:, 0 : t1 - t0, :],
                                 func=AF.Exp, scale=-0.5)
            if mrsneg is not None:
                # -mean * rsqrt, for the ACT-side LN apply
                nc.vector.scalar_tensor_tensor(
                    out=mrsneg[:, t0:t1, :], in0=mv[:, t0:t1, 0:1],
                    scalar=-1.0, in1=rs[:, t0:t1, :],
                    op0=ALU.mult, op1=ALU.mult,
                )

        def ln_apply_transpose(mv, rs, t, mrsneg=None):
            h = hlnp.tile([128, D], bf16, tag="hln")
            if mrsneg is None:
                nc.vector.tensor_scalar(
                    out=h, in0=x_t[:, t, :],
                    scalar1=mv[:, t, 0:1], scalar2=rs[:, t, :],
                    op0=ALU.subtract, op1=ALU.mult,
                )
            else:
                nc.scalar.activation(
                    out=h, in_=x_t[:, t, :], func=AF.Identity,
                    scale=rs[:, t, :], bias=mrsneg[:, t, :],
                )
            po = pq.tile([128, D], bf16, tag="po", bufs=2)
            pov = po.rearrange("p (db c) -> p db c", db=DB)
            for db in range(DB):
                nc.tensor.transpose(
                    pov[:, db, :], h[:, db * 128 : (db + 1) * 128], ident
                )
            nc.scalar.activation(
                out=hfm[:, :, t * 128 : (t + 1) * 128], in_=pov, func=AF.Identity
            )

        deferred_w2 = []

        for li in range(n_layers):
            # ---- prompt replacement for rows landing in tiles 0..7
            # (tiles 8/9 carry the previous layer's deferred W2 residuals and
            #  are patched after those run)
            def prompt_dma(b):
                t0 = b * S + 1
                p0, tl = t0 % 128, t0 // 128
                nc.sync.dma_start(out=x_t[p0 : p0 + NP, tl, :], in_=PR[li - 1, b])

            if li > 0:
                for b in range(BSH - 2):
                    prompt_dma(b)
            bq_t = bp.tile([128, DB], f32, tag="bq")
            nc.sync.dma_start(out=bq_t, in_=BQ[li])
            bk_t = bp.tile([128, DB], f32, tag="bk")
            nc.sync.dma_start(out=bk_t, in_=BK[li])
            b1_t = bp.tile([128, FB], f32, tag="b1")
            nc.sync.dma_start(out=b1_t, in_=B1[li])
            brow_t = bp.tile([1, 2, D], bf16, tag="brow")
            nc.sync.dma_start(out=brow_t, in_=BROW[li])
            wq_t = wpool.tile([128, DB, D], bf16, tag="wq")
            nc.sync.dma_start(out=wq_t, in_=WQ[li].rearrange("(kb p) m -> p kb m", p=128))
            wk_t = wpool.tile([128, DB, D], bf16, tag="wk")
            nc.sync.dma_start(out=wk_t, in_=WK[li].rearrange("(kb p) m -> p kb m", p=128))
            wv_t = wpool.tile([128, DB, D], bf16, tag="wv")
            nc.sync.dma_start(out=wv_t, in_=WV[li].rearrange("(kb p) m -> p kb m", p=128))
            wo_t = wpool.tile([128, DB, D], bf16, tag="wo")
            nc.sync.dma_start(out=wo_t, in_=WO[li].rearrange("(kb p) m -> p kb m", p=128))
            w1_t = wpool.tile([128, DB, FF], bf16, tag="w1")
            nc.sync.dma_start(out=w1_t, in_=W1[li].rearrange("(kb p) m -> p kb m", p=128))
            w2_t = wpool.tile([128, FB, D], bf16, tag="w2")
            nc.sync.dma_start(out=w2_t, in_=W2[li].rearrange("(kb p) m -> p kb m", p=128))

            def qk_chunk(c):
                c0, cw = QKCH[c]
                for m in range(DB):
                    ps = pq.tile([128, D], f32, tag="pq")
                    for k in range(DB):
                        nc.tensor.matmul(
                            ps[:, 0:cw],
                            wq_t[:, k, m * 128 : (m + 1) * 128],
                            hfm[:, k, c0 : c0 + cw],
                            start=(k == 0), stop=(k == DB - 1),
                        )
                    nc.scalar.activation(
                        out=q_fm[:, m, c0 : c0 + cw], in_=ps[:, 0:cw],
                        func=AF.Identity, bias=bq_t[:, m : m + 1],
                    )
                for m in range(DB):
                    ps = pq.tile([128, D], f32, tag="pq")
                    for k in range(DB):
                        nc.tensor.matmul(
                            ps[:, 0:cw],
                            wk_t[:, k, m * 128 : (m + 1) * 128],
                            hfm[:, k, c0 : c0 + cw],
                            start=(k == 0), stop=(k == DB - 1),
                        )
                    nc.vector.tensor_scalar_add(
                        out=k_fm[:, m, c0 : c0 + cw], in0=ps[:, 0:cw],
                        scalar1=bk_t[:, m : m + 1],
                    )
                for b in V_READY[c]:
                    bs = b * S
                    ps = pq.tile([128, D], f32, tag="pq")
                    for k in range(DB):
                        nc.tensor.matmul(
                            ps[0:P, :],
                            hfm[:, k, bs : bs + S],
                            wv_t[:, k, :],
                            start=(k == 0), stop=(k == DB - 1),
                        )
                    nc.vector.tensor_copy(
                        out=v_tm[0:P, b, :, 0:DH],
                        in_=ps[0:P, :].rearrange("p (h d) -> p h d", h=H),
                    )

            # ---- phase L: LN1 + Q/K/V, deferred W2 tail of previous layer
            mv1 = statp.tile([128, NTILES, 2], f32, tag="mv1")
            rs1 = statp.tile([128, NTILES, 1], f32, tag="rs1")
            for t in range(0, 5):
                ln_stats(mv1, t)
            ln_rsqrt_batch(mv1, rs1, 0, 5)
            for t in range(0, 4):
                ln_apply_transpose(mv1, rs1, t)
                if deferred_w2 and t < 2:
                    deferred_w2.pop(0)()
                    if not deferred_w2 and li > 0:
                        prompt_dma(BSH - 2)
                        prompt_dma(BSH - 1)
            qk_chunk(0)
            for t in range(5, 10):
                ln_stats(mv1, t)
            ln_rsqrt_batch(mv1, rs1, 5, 10)
            for t in range(4, 8):
                ln_apply_transpose(mv1, rs1, t)
            qk_chunk(1)
            for t in range(8, 10):
                ln_apply_transpose(mv1, rs1, t)
            qk_chunk(2)

            # ---- phase A1: scores + softmax numerators for all sequences
            pts = []
            for b in range(BSH):
                bs = b * S
                sc = pbig.tile([128, 1024], f32, tag="pbig")
                scv = sc.rearrange("p (e d q) -> p e d q", e=2, q=128)
                for dbl in range(DB):
                    nc.tensor.matmul(
                        scv[0:P, 0, dbl, 0:P],
                        k_fm[0:64, dbl, bs : bs + S],
                        q_fm[0:64, dbl, bs : bs + S],
                        start=True, stop=True,
                    )
                    nc.tensor.matmul(
                        scv[0:P, 1, dbl, 0:P],
                        k_fm[64:128, dbl, bs : bs + S],
                        q_fm[64:128, dbl, bs : bs + S],
                        start=True, stop=True,
                    )
                pt = ptp.tile([128, 2, DB, 80], bf16, tag="pt")
                for e in range(2):
                    nc.scalar.activation(
                        out=pt[0:P, e, :, 0:P], in_=scv[0:P, e, :, 0:P],
                        func=AF.Exp,
                    )
                    nc.vector.tensor_mul(
                        pt[0:P, e, :, 0:P], pt[0:P, e, :, 0:P],
                        causal[0:P, 0:P].unsqueeze(1).broadcast_to((P, DB, P)),
                    )
                pts.append(pt)

            # ---- phase A2: AV + output transposes, interleaved with Wo/LN2/W1
            mv2 = statp.tile([128, NTILES, 2], f32, tag="mv2")
            rs2 = statp.tile([128, NTILES, 1], f32, tag="rs2")
            wo_done = 0
            w1_early = 0

            def wo_tile(t):
                ps = pq.tile([128, D], f32, tag="pq")
                for k in range(DB):
                    nc.tensor.matmul(
                        ps[:, :],
                        o_fm[:, k, t * 128 : (t + 1) * 128],
                        wo_t[:, k, :],
                        start=(k == 0), stop=False,
                    )
                nc.tensor.matmul(
                    ps[:, :], ones_row[0:1, :], brow_t[0:1, 0, :],
                    start=False, stop=True,
                )
                nc.vector.tensor_add(x_t[:, t, :], x_t[:, t, :], ps[:, :])
                ln_stats(mv2, t)

            def w1_block(c0, cw, m):
                ps = pq.tile([128, D], f32, tag="pq")
                for k in range(DB):
                    nc.tensor.matmul(
                        ps[:, 0:cw],
                        w1_t[:, k, m * 128 : (m + 1) * 128],
                        hfm[:, k, c0 : c0 + cw],
                        start=(k == 0), stop=(k == DB - 1),
                    )
                nc.scalar.activation(
                    out=h_mlp[:, m, c0 : c0 + cw], in_=ps[:, 0:cw],
                    func=AF.Gelu_apprx_sigmoid, bias=b1_t[:, m : m + 1],
                )

            for b in range(BSH):
                bs = b * S
                pt = pts[b]
                otf = pbig.tile([128, 1024], f32, tag="pbig")
                ot = otf.rearrange("p (h c) -> p h c", c=128)
                for h in range(H):
                    nc.tensor.matmul(
                        ot[0:P, h, 0 : DH + 1],
                        pt[0:P, h % 2, h // 2, 0:P],
                        v_tm[0:P, b, h, :],
                        start=True, stop=True,
                    )
                rec = statp.tile([128, H], f32, tag="rec")
                nc.vector.reciprocal(
                    out=rec[0:P, :].unsqueeze(2), in_=ot[0:P, :, DH : DH + 1]
                )
                osb = osbp.tile([128, D], bf16, tag="osb")
                nc.vector.tensor_mul(
                    osb[0:P, :].rearrange("p (h d) -> p h d", h=H),
                    ot[0:P, :, 0:DH],
                    rec[0:P, :].unsqueeze(2).broadcast_to((P, H, DH)),
                )
                po = pq.tile([128, D], bf16, tag="po", bufs=2)
                pov = po.rearrange("p (db c) -> p db c", db=DB)
                for db in range(DB):
                    nc.tensor.transpose(
                        pov[:, db, 0:P], osb[0:P, db * 128 : (db + 1) * 128],
                        ident[0:P, 0:P],
                    )
                nc.scalar.activation(
                    out=o_fm[:, :, bs : bs + S], in_=pov[:, :, 0:P],
                    func=AF.Identity,
                )

                while (wo_done + 1) * 128 <= (b + 1) * S:
                    wo_tile(wo_done)
                    wo_done += 1
                    if wo_done == 5:
                        ln_rsqrt_batch(mv2, rs2, 0, 5)
                        for t in range(0, 5):
                            ln_apply_transpose(mv2, rs2, t)
                for _ in range(2):
                    if wo_done >= 5 and w1_early < W1_EARLY:
                        w1_block(W1CH[0][0], W1CH[0][1], w1_early)
                        w1_early += 1

            for t in range(wo_done, NTILES):
                wo_tile(t)
            ln_rsqrt_batch(mv2, rs2, 5, NTILES)
            for t in range(5, NTILES):
                ln_apply_transpose(mv2, rs2, t)

            # ---- phase M: remaining W1 + W2 (last two tiles deferred)
            def w2_tile(t, w2_ref, brow_ref):
                def emit():
                    ps = pq.tile([128, D], f32, tag="pq")
                    for k in range(FB):
                        nc.tensor.matmul(
                            ps[:, :],
                            h_mlp[:, k, t * 128 : (t + 1) * 128],
                            w2_ref[:, k, :],
                            start=(k == 0), stop=False,
                        )
                    nc.tensor.matmul(
                        ps[:, :], ones_row[0:1, :], brow_ref[0:1, 1, :],
                        start=False, stop=True,
                    )
                    nc.vector.tensor_add(x_t[:, t, :], x_t[:, t, :], ps[:, :])
                return emit

            W2T = [range(0, 4), range(4, 8), range(8, 10)]
            last = li == n_layers - 1
            for ci, (c0, cw) in enumerate(W1CH):
                for m in range(w1_early if ci == 0 else 0, FB):
                    w1_block(c0, cw, m)
                for t in W2T[ci]:
                    if ci == 2 and not last:
                        deferred_w2.append(w2_tile(t, w2_t, brow_t))
                    else:
                        w2_tile(t, w2_t, brow_t)()

        nc.sync.dma_start(out=OUT[:, :, :], in_=x_t)
    return nc


_NC_CACHE = None
_LAST_IN_MAPS = None


def _get_nc():
    global _NC_CACHE
    if _NC_CACHE is None:
        _NC_CACHE = build_program()
    return _NC_CACHE


def kernel(text_tokens, attn_mask, g_prompt, s_prompt, token_emb, pos_emb,
           ln1_g, ln1_b, Wq, bq, Wk, bk, Wv, bv, Wo, bo,
           ln2_g, ln2_b, W1, b1, W2, b2, lnf_g, lnf_b):
    text_tokens = np.asarray(text_tokens)
    attn_mask = np.asarray(attn_mask)
    assert np.all(np.asarray(attn_mask) == 1), "kernel assumes all-ones attn_mask"
    fp = lambda a: np.asarray(a, dtype=np.float32)
    g_prompt, s_prompt = fp(g_prompt), fp(s_prompt)
    token_emb, pos_emb = fp(token_emb), fp(pos_emb)
    ln1_g, ln1_b, ln2_g, ln2_b = fp(ln1_g), fp(ln1_b), fp(ln2_g), fp(ln2_b)
    Wq, Wk, Wv, Wo, W1, W2 = fp(Wq), fp(Wk), fp(Wv), fp(Wo), fp(W1), fp(W2)
    bq, bk, bv, bo, b1, b2 = fp(bq), fp(bk), fp(bv), fp(bo), fp(b1), fp(b2)
    lnf_g, lnf_b = fp(lnf_g), fp(lnf_b)

    # ---- host-side input prep
    emb = token_emb[text_tokens]                                  # [B, T, D]
    x0 = np.concatenate([emb[:, :1], g_prompt[:, 0], emb[:, 1:]], axis=1)
    x0 = x0 + pos_emb[None, :S]                                   # [B, S, D]

    # fold LN gains/scale into weights; fold bv through Wo into a bias row
    wq_e = _bf(ln1_g[:, :, None] * Wq * SCALE)
    wk_e = _bf(ln1_g[:, :, None] * Wk)
    wv_e = _bf(ln1_g[:, :, None] * Wv)
    wo_e = _bf(Wo)
    w1_e = _bf(ln2_g[:, :, None] * W1)
    w2_e = _bf(W2)
    bq_e = (bq + np.einsum("ld,ldm->lm", ln1_b, Wq)) * SCALE      # [NL, D]
    bk_e = bk + np.einsum("ld,ldm->lm", ln1_b, Wk)
    bv_e = bv + np.einsum("ld,ldm->lm", ln1_b, Wv)
    b1_e = b1 + np.einsum("ld,ldm->lm", ln2_b, W1)
    crow = np.einsum("ld,ldm->lm", bv_e, Wo) + bo                 # bv@Wo + bo
    bq_dev = _f32(bq_e.reshape(NL, DB, 128).transpose(0, 2, 1))
    bk_dev = _f32(bk_e.reshape(NL, DB, 128).transpose(0, 2, 1))
    b1_dev = _f32(b1_e.reshape(NL, FB, 128).transpose(0, 2, 1))
    brow = _bf(np.stack([crow, np.broadcast_to(b2, (NL, D))], axis=1))

    causalT = np.zeros((P, 128), np.float32)                      # [tk, tq]
    causalT[:, :P] = np.triu(np.ones((P, P), np.float32))
    ident = _bf(np.eye(128, dtype=np.float32))

    in_maps = []
    for c in range(NCORES):
        sl = slice(c * BSH, (c + 1) * BSH)
        flat = x0[sl].reshape(NT, D)
        flat = np.concatenate([flat, np.zeros((NTILES * 128 - NT, D), np.float32)])
        x0_dev = _f32(flat.reshape(NTILES, 128, D).transpose(1, 0, 2))
        prompts = np.stack(
            [g_prompt[sl, i] if i < DG else s_prompt[sl, i - (NL - DS)]
             for i in range(1, NL)]
        )
        in_maps.append({
            "x0": x0_dev, "prompts": _f32(prompts),
            "wq": wq_e, "wk": wk_e, "wv": wv_e, "wo": wo_e,
            "w1": w1_e, "w2": w2_e,
            "bq": bq_dev, "bk": bk_dev, "b1": b1_dev, "brow": brow,
            "causal": _bf(causalT), "ident": ident,
        })

    nc = _get_nc()
    global _LAST_IN_MAPS
    _LAST_IN_MAPS = in_maps
    res = run_bass_kernel_spmd(nc, in_maps, core_ids=list(range(NCORES)))

    # ---- host-side epilogue: final LN + EOT gather
    idx = np.argmax(text_tokens, axis=-1) + NP                    # [B]
    out = np.empty((B, D), np.float32)
    for c in range(NCORES):
        xr = res.results[c]["out"].transpose(1, 0, 2).reshape(NTILES * 128, D)
        for b in range(BSH):
            row = xr[(b * S) + idx[c * BSH + b]]
            m = row.mean()
            v = ((row - m) ** 2).mean()
            out[c * BSH + b] = (row - m) / np.sqrt(v + EPS) * lnf_g + lnf_b
    return out
